# revision 6
# baseline (speedup 1.0000x reference)
"""Trainium2 Bass kernel for nn_CustomConvolve (2x2 locally-connected conv).

Reference computation (per image):
  out[w, h] = x[w-1,h-1]*W0(w,h) + x[w-1,h]*W1(w,h)
            + x[w,  h-1]*W2(w,h) + x[w,  h]*W3(w,h) + bias(w,h)
  for w,h in [1, 510]; out row 0 and col 0 are zero.
  Weight index: idx = 511*w + h into weights[261121, 4] / bias[261121].

Sharding: data-parallel over batch. 16 batches / 8 cores = 2 per core;
each core processes 32 (b,c) images of 512x512. weights/bias replicated.

v2 layout: one whole image per SBUF tile, partition p holds x rows
4p..4p+3 (8KB contiguous DMA chunks). For output row w = 4p+r:
  r in {1,2,3} ("easy"): both x rows (w-1, w) live in partition p, so all
    four products are partition-aligned DVE fp16 tensor_tensor ops (2x
    perf mode) and accumulate via plain-identity matmuls into PSUM.
  r == 0 ("hard"): x row w-1 is partition p-1's last row. Products for
    those terms are computed in partition p-1 and accumulated through a
    shifted identity (lhsT[p, j] = delta(p, j-1)) on the PE; the x-row-w
    terms and bias use an identity with [0,0]=0 so out row 0 stays zero.
  PSUM tile [128, 4, 512] = 4 banks (one per r); x2 pool bufs = all 8.
  Scalar engine: f32->fp16 casts of x (two alignments so every DVE read
  is 4B-aligned) and PSUM->SBUF evacuation. DMA triggers ride the
  hardware DGE queues (sync engine for loads, scalar for stores);
  weights/bias load+repack once per core.
"""

import os
import sys

for _p in ("/opt/trn_rl_repo",):
    if _p not in sys.path and os.path.isdir(_p):
        sys.path.append(_p)

import numpy as np

import concourse.bass as bass
import concourse.mybir as mybir
from concourse import bacc
from concourse.bass_utils import run_bass_kernel_spmd
from concourse.masks import make_identity
from concourse.tile import TileContext

N_CORES = 8
B, C, W, H = 16, 16, 512, 512
B_PER_CORE = B // N_CORES          # 2
IMGS = B_PER_CORE * C              # 32 images per core
OW, OH = W - 1, H - 1              # 511, 511
NW = W - 1                         # weight-grid row pitch (511)
NVAL = 510                         # valid output rows/cols: 1..510
R = 4                              # x rows per partition
ROWELEMS = NW * 4                  # 2044 packed weight elems per grid row

F32 = mybir.dt.float32
F16 = mybir.dt.float16


def _build():
    nc = bacc.Bacc("TRN2", debug=False, target_bir_lowering=False, num_swdge_queues=4)

    x_d = nc.dram_tensor("x", [IMGS, W, H], F32, kind="ExternalInput")
    w_d = nc.dram_tensor("weights", [NW * NW, 4], F32, kind="ExternalInput")
    b_d = nc.dram_tensor("bias", [NW * NW], F32, kind="ExternalInput")
    o_d = nc.dram_tensor("out", [IMGS, OW, OH], F32, kind="ExternalOutput")

    with TileContext(nc) as tc:
        with (
            tc.tile_pool(name="const", bufs=1) as cpool,
            tc.tile_pool(name="wts", bufs=1) as wpool,
            tc.tile_pool(name="xf", bufs=2) as xfpool,
            tc.tile_pool(name="x16", bufs=2) as xpool,
            tc.tile_pool(name="prod", bufs=2) as mpool,
            tc.tile_pool(name="ot", bufs=3) as opool,
            tc.tile_pool(name="psum", bufs=2, space="PSUM") as psum_pool,
        ):
            # --- identities (fp16 lhsT variants) ---
            identf = cpool.tile([128, 128], F32)
            make_identity(nc, identf)
            id16 = cpool.tile([128, 128], F16)
            nc.vector.tensor_copy(out=id16, in_=identf)
            # identity with [0,0]=0: kills out row 0 contributions
            idz = cpool.tile([128, 128], F16)
            nc.vector.tensor_copy(out=idz, in_=identf)
            nc.vector.memset(idz[0:1, 0:1], 0.0)
            # shifted identity: sh[p, j] = 1 iff j == p+1 (psum[j] += rhs[j-1])
            sh = cpool.tile([128, 128], F16)
            nc.vector.memset(sh, 0.0)
            nc.vector.tensor_copy(out=sh[0:127, 1:128], in_=identf[0:127, 0:127])

            # --- weights/bias: load once, repack to fp16 planes ---
            # wq[k][r-1]: w_k(4p+r, h) for easy rows r=1..3
            wq = wpool.tile([128, 4, 3, NVAL], F16)
            # wv[k]: w_{2+k}(4p, h);  wu[k]: w_k(4(p+1), h)  (hard row terms)
            wv = wpool.tile([128, 2, NVAL], F16)
            wu = wpool.tile([127, 2, NVAL], F16)
            # bias planes
            be = wpool.tile([128, 3, NVAL], F16)
            bh = wpool.tile([128, NVAL], F16)

            with tc.tile_pool(name="wstage", bufs=1) as spool:
                # packed weight rows 4p..4p+3 per partition; partition 127
                # only has 3 valid rows (grid rows 508..510), tail is junk
                # that is never read for a shipped output.
                wt_r = spool.tile([128, R, NW, 4], F32)
                nc.gpsimd.dma_start(
                    out=wt_r[0:127],
                    in_=bass.AP(w_d, 0, [[R * ROWELEMS, 127], [1, R * ROWELEMS]]),
                )
                nc.gpsimd.dma_start(
                    out=wt_r[127:128, 0:3],
                    in_=bass.AP(w_d, 127 * R * ROWELEMS, [[1, 3 * ROWELEMS]]),
                )
                # weight rows 4(p+1) for the shifted (hard-u) terms
                wuv_r = spool.tile([127, NW, 4], F32)
                nc.gpsimd.dma_start(
                    out=wuv_r,
                    in_=bass.AP(w_d, R * ROWELEMS, [[R * ROWELEMS, 127], [1, ROWELEMS]]),
                )
                # bias rows 4p..4p+3
                ball_r = spool.tile([128, R, NW], F32)
                nc.gpsimd.dma_start(
                    out=ball_r[0:127],
                    in_=bass.AP(b_d, 0, [[R * NW, 127], [1, R * NW]]),
                )
                nc.gpsimd.dma_start(
                    out=ball_r[127:128, 0:3],
                    in_=bass.AP(b_d, 127 * R * NW, [[1, 3 * NW]]),
                )

                for k in range(4):
                    # wq[k][r-1][h] = w_k(4p+r, 1+h), r=1..3
                    nc.vector.tensor_copy(
                        out=wq[:, k], in_=wt_r[:, 1:4, 1 : 1 + NVAL, k]
                    )
                for k in range(2):
                    nc.vector.tensor_copy(
                        out=wv[:, k], in_=wt_r[:, 0, 1 : 1 + NVAL, 2 + k]
                    )
                    nc.vector.tensor_copy(
                        out=wu[:, k], in_=wuv_r[:, 1 : 1 + NVAL, k]
                    )
                nc.vector.tensor_copy(out=be, in_=ball_r[:, 1:4, 1 : 1 + NVAL])
                nc.vector.tensor_copy(out=bh, in_=ball_r[:, 0, 1 : 1 + NVAL])

            # --- per-image pipeline ---
            for img in range(IMGS):
                xf = xfpool.tile([128, R, H], F32, tag="xf")
                nc.sync.dma_start(
                    out=xf,
                    in_=bass.AP(x_d, img * W * H, [[R * H, 128], [1, R * H]]),
                )
                # fp16 copies at two alignments so every DVE read below is
                # 4B-aligned (2x perf mode needs step 1 + 4B alignment).
                # Only cols 0..509 of each row-strip are ever read.
                xa = xpool.tile([128, R, H], F16, tag="xa")
                nc.scalar.copy(
                    out=xa[:, :, 0:NVAL], in_=xf[:, :, 0:NVAL]
                )
                xb = xpool.tile([128, R, H], F16, tag="xb")
                nc.scalar.copy(
                    out=xb[:, :, 0:NVAL], in_=xf[:, :, 1 : 1 + NVAL]
                )

                # easy products me[k][r-1] = x_term_k(4p+r, :) * wq[k][r-1]
                me = mpool.tile([128, 4, 3, NVAL], F16, tag="me")
                nc.vector.tensor_mul(
                    out=me[:, 0], in0=xa[:, 0:3, 0:NVAL], in1=wq[:, 0]
                )
                nc.vector.tensor_mul(
                    out=me[:, 1], in0=xb[:, 0:3, 0:NVAL], in1=wq[:, 1]
                )
                nc.vector.tensor_mul(
                    out=me[:, 2], in0=xa[:, 1:4, 0:NVAL], in1=wq[:, 2]
                )
                nc.vector.tensor_mul(
                    out=me[:, 3], in0=xb[:, 1:4, 0:NVAL], in1=wq[:, 3]
                )
                # hard products: mu_k at partition p-1 (x row 4p-1), mv_k at p
                mh = mpool.tile([128, 4, NVAL], F16, tag="mh")
                nc.vector.tensor_mul(
                    out=mh[0:127, 0], in0=xa[0:127, 3, 0:NVAL], in1=wu[:, 0]
                )
                nc.vector.tensor_mul(
                    out=mh[0:127, 1], in0=xb[0:127, 3, 0:NVAL], in1=wu[:, 1]
                )
                nc.vector.tensor_mul(
                    out=mh[:, 2], in0=xa[:, 0, 0:NVAL], in1=wv[:, 0]
                )
                nc.vector.tensor_mul(
                    out=mh[:, 3], in0=xb[:, 0, 0:NVAL], in1=wv[:, 1]
                )

                # PSUM accumulate: bank r holds out rows 4p+r, cols 1..510
                acc = psum_pool.tile([128, 4, 512], F32)
                a0 = acc[:, 0, 0:NVAL]
                nc.tensor.matmul(a0, sh[0:127, :], mh[0:127, 0], start=True, stop=False)
                nc.tensor.matmul(a0, sh[0:127, :], mh[0:127, 1], start=False, stop=False)
                nc.tensor.matmul(a0, idz, mh[:, 2], start=False, stop=False)
                nc.tensor.matmul(a0, idz, mh[:, 3], start=False, stop=False)
                nc.tensor.matmul(a0, idz, bh, start=False, stop=True)
                for r in range(1, 4):
                    ar = acc[:, r, 0:NVAL]
                    for k in range(4):
                        nc.tensor.matmul(
                            ar, id16, me[:, k, r - 1], start=(k == 0), stop=False
                        )
                    nc.tensor.matmul(ar, id16, be[:, r - 1], start=False, stop=True)

                # evacuate + zero col 0, ship
                ot = opool.tile([128, R, OH], F32, tag="ot")
                nc.vector.memset(ot[:, :, 0:1], 0.0)
                nc.scalar.copy(out=ot[:, :, 1:OH], in_=acc[:, :, 0:NVAL])
                nc.scalar.dma_start(
                    out=bass.AP(o_d, img * OW * OH, [[R * OH, 127], [1, R * OH]]),
                    in_=ot[0:127, :, :],
                )
                nc.scalar.dma_start(
                    out=bass.AP(
                        o_d, img * OW * OH + 508 * OH, [[1, 3 * OH]]
                    ),
                    in_=ot[127:128, 0:3, :],
                )

    nc.finalize()
    return nc


_CACHE = {}


def _get_nc():
    if "nc" not in _CACHE:
        _CACHE["nc"] = _build()
    return _CACHE["nc"]


def kernel(x, weights, bias):
    assert x.shape == (B, C, W, H) and x.dtype == np.float32
    nc = _get_nc()

    in_maps = []
    for i in range(N_CORES):
        shard = np.ascontiguousarray(
            x[i * B_PER_CORE : (i + 1) * B_PER_CORE].reshape(IMGS, W, H)
        )
        in_maps.append({"x": shard, "weights": weights, "bias": bias})

    trace = os.environ.get("BASS_TRACE") == "1"
    res = run_bass_kernel_spmd(
        nc, in_maps, core_ids=list(range(N_CORES)), trace=trace
    )
    kernel.last_exec_time_ns = res.exec_time_ns
    kernel.last_results = res

    out = np.empty((B, C, OW, OH), dtype=np.float32)
    for i in range(N_CORES):
        out[i * B_PER_CORE : (i + 1) * B_PER_CORE] = res.results[i]["out"].reshape(
            B_PER_CORE, C, OW, OH
        )
    # Row 0 / col 0 are zero by definition; enforce host-side.
    out[:, :, 0, :] = 0.0
    out[:, :, :, 0] = 0.0
    return out


# revision 7
# speedup vs baseline: 1.7342x; 1.7342x over previous
"""Trainium2 Bass kernel for nn_CustomConvolve (2x2 locally-connected conv).

Reference computation (per image):
  out[w, h] = x[w-1,h-1]*W0(w,h) + x[w-1,h]*W1(w,h)
            + x[w,  h-1]*W2(w,h) + x[w,  h]*W3(w,h) + bias(w,h)
  for w,h in [1, 510]; out row 0 and col 0 are zero.
  Weight index: idx = 511*w + h into weights[261121, 4] / bias[261121].

Sharding: data-parallel over batch. 16 batches / 8 cores = 2 per core;
each core processes 32 (b,c) images of 512x512. weights/bias replicated.

Layout: one whole image per SBUF tile, partition p holds x rows
4p..4p+3 (8KB contiguous DMA chunks). For output row w = 4p+r:
  r in {1,2,3} ("easy"): both x rows (w-1, w) live in partition p, so all
    four products are partition-aligned DVE fp16 tensor_tensor ops (2x
    perf mode), pair-summed on DVE, then accumulated via plain-identity
    matmuls into PSUM.
  r == 0 ("hard"): x row w-1 is partition p-1's last row. Products for
    those terms are computed in partition p-1 and accumulated through a
    shifted identity (lhsT[p, j] = delta(p, j-1)) on the PE; the x-row-w
    terms and bias use an identity with [0,0]=0 so out row 0 stays zero.
  PSUM tile [128, 4, 512] = 4 banks (one per r), pool bufs=2 -> 8 banks.
  Scalar engine: f32->fp16 casts of x (two alignments so every DVE read
  is 4B-aligned) and PSUM->SBUF evacuation.
  All bulk DMA rides gpsimd SWDGE queues (spread across all 16 DMA
  engines); x loads are triggered 2+ images ahead of out stores so the
  single trigger stream never stalls loads behind compute.
"""

import os
import sys

for _p in ("/opt/trn_rl_repo",):
    if _p not in sys.path and os.path.isdir(_p):
        sys.path.append(_p)

import numpy as np

import concourse.bass as bass
import concourse.mybir as mybir
from concourse import bacc
from concourse.bass_utils import run_bass_kernel_spmd
from concourse.masks import make_identity
from concourse.tile import TileContext

N_CORES = 8
B, C, W, H = 16, 16, 512, 512
B_PER_CORE = B // N_CORES          # 2
IMGS = B_PER_CORE * C              # 32 images per core
OW, OH = W - 1, H - 1              # 511, 511
NW = W - 1                         # weight-grid row pitch (511)
NVAL = 510                         # valid output rows/cols: 1..510
R = 4                              # x rows per partition
ROWELEMS = NW * 4                  # 2044 packed weight elems per grid row
PREFETCH = 3                       # x loads triggered this many images ahead

F32 = mybir.dt.float32
F16 = mybir.dt.float16


def _build():
    nc = bacc.Bacc("TRN2", debug=False, target_bir_lowering=False, num_swdge_queues=4)

    x_d = nc.dram_tensor("x", [IMGS, W, H], F32, kind="ExternalInput")
    w_d = nc.dram_tensor("weights", [NW * NW, 4], F32, kind="ExternalInput")
    b_d = nc.dram_tensor("bias", [NW * NW], F32, kind="ExternalInput")
    o_d = nc.dram_tensor("out", [IMGS, OW, OH], F32, kind="ExternalOutput")

    with TileContext(nc) as tc:
        with (
            tc.tile_pool(name="const", bufs=1) as cpool,
            tc.tile_pool(name="wts", bufs=1) as wpool,
            tc.tile_pool(name="xf", bufs=2 + PREFETCH) as xfpool,
            tc.tile_pool(name="x16", bufs=2) as xpool,
            tc.tile_pool(name="prod", bufs=2) as mpool,
            tc.tile_pool(name="ot", bufs=3) as opool,
            tc.tile_pool(name="psum", bufs=2, space="PSUM") as psum_pool,
        ):
            # --- identities (fp16 lhsT variants) ---
            identf = cpool.tile([128, 128], F32)
            make_identity(nc, identf)
            id16 = cpool.tile([128, 128], F16)
            nc.vector.tensor_copy(out=id16, in_=identf)
            # identity with [0,0]=0: kills out row 0 contributions
            idz = cpool.tile([128, 128], F16)
            nc.vector.tensor_copy(out=idz, in_=identf)
            nc.vector.memset(idz[0:1, 0:1], 0.0)
            # shifted identity: sh[p, j] = 1 iff j == p+1 (psum[j] += rhs[j-1])
            sh = cpool.tile([128, 128], F16)
            nc.vector.memset(sh, 0.0)
            nc.vector.tensor_copy(out=sh[0:127, 1:128], in_=identf[0:127, 0:127])

            # --- weights/bias: load once, repack to fp16 planes ---
            # wq[k][r-1]: w_k(4p+r, h) for easy rows r=1..3
            wq = wpool.tile([128, 4, 3, NVAL], F16)
            # wv[k]: w_{2+k}(4p, h);  wu[k]: w_k(4(p+1), h)  (hard row terms)
            wv = wpool.tile([128, 2, NVAL], F16)
            wu = wpool.tile([127, 2, NVAL], F16)
            # bias planes
            be = wpool.tile([128, 3, NVAL], F16)
            bh = wpool.tile([128, NVAL], F16)

            with tc.tile_pool(name="wstage", bufs=1) as spool:
                # packed weight rows 4p..4p+3 per partition; partition 127
                # only has 3 valid rows (grid rows 508..510), tail is junk
                # that never reaches a shipped output.
                wt_r = spool.tile([128, R, NW, 4], F32)
                nc.gpsimd.dma_start(
                    out=wt_r[0:127],
                    in_=bass.AP(w_d, 0, [[R * ROWELEMS, 127], [1, R * ROWELEMS]]),
                )
                nc.gpsimd.dma_start(
                    out=wt_r[127:128, 0:3],
                    in_=bass.AP(w_d, 127 * R * ROWELEMS, [[1, 3 * ROWELEMS]]),
                )
                # weight rows 4(p+1) for the shifted (hard-u) terms
                wuv_r = spool.tile([127, NW, 4], F32)
                nc.gpsimd.dma_start(
                    out=wuv_r,
                    in_=bass.AP(w_d, R * ROWELEMS, [[R * ROWELEMS, 127], [1, ROWELEMS]]),
                )
                # bias rows 4p..4p+3
                ball_r = spool.tile([128, R, NW], F32)
                nc.gpsimd.dma_start(
                    out=ball_r[0:127],
                    in_=bass.AP(b_d, 0, [[R * NW, 127], [1, R * NW]]),
                )
                nc.gpsimd.dma_start(
                    out=ball_r[127:128, 0:3],
                    in_=bass.AP(b_d, 127 * R * NW, [[1, 3 * NW]]),
                )

                for k in range(4):
                    # wq[k][r-1][h] = w_k(4p+r, 1+h), r=1..3
                    nc.vector.tensor_copy(
                        out=wq[:, k], in_=wt_r[:, 1:4, 1 : 1 + NVAL, k]
                    )
                for k in range(2):
                    nc.vector.tensor_copy(
                        out=wv[:, k], in_=wt_r[:, 0, 1 : 1 + NVAL, 2 + k]
                    )
                    nc.vector.tensor_copy(
                        out=wu[:, k], in_=wuv_r[:, 1 : 1 + NVAL, k]
                    )
                nc.vector.tensor_copy(out=be, in_=ball_r[:, 1:4, 1 : 1 + NVAL])
                nc.vector.tensor_copy(out=bh, in_=ball_r[:, 0, 1 : 1 + NVAL])

            # --- per-image pipeline ---
            def load_x(img):
                xf = xfpool.tile([128, R, H], F32, tag="xf", name=f"xf{img}")
                nc.gpsimd.dma_start(
                    out=xf,
                    in_=bass.AP(x_d, img * W * H, [[R * H, 128], [1, R * H]]),
                )
                return xf

            xf_q = [load_x(i) for i in range(min(PREFETCH, IMGS))]

            for img in range(IMGS):
                xf = xf_q.pop(0)
                if img + PREFETCH < IMGS:
                    xf_q.append(load_x(img + PREFETCH))
                # fp16 copies at two alignments so every DVE read below is
                # 4B-aligned (2x perf mode needs step 1 + 4B alignment).
                # Contiguous full-width casts keep ACTIVATE in 2x mode.
                xa = xpool.tile([128, R, H], F16, tag="xa")
                nc.scalar.copy(
                    out=xa.tensor.reshape([128, R * H])[:, :],
                    in_=xf.tensor.reshape([128, R * H])[:, :],
                )
                xb = xpool.tile([128, R, H], F16, tag="xb")
                nc.scalar.copy(
                    out=xb.tensor.reshape([128, R * H])[:, 0 : R * H - 2],
                    in_=xf.tensor.reshape([128, R * H])[:, 1 : R * H - 1],
                )

                # easy products me[k][r-1] = x_term_k(4p+r, :) * wq[k][r-1]
                me = mpool.tile([128, 4, 3, NVAL], F16, tag="me")
                nc.vector.tensor_mul(
                    out=me[:, 0], in0=xa[:, 0:3, 0:NVAL], in1=wq[:, 0]
                )
                nc.vector.tensor_mul(
                    out=me[:, 1], in0=xb[:, 0:3, 0:NVAL], in1=wq[:, 1]
                )
                nc.vector.tensor_mul(
                    out=me[:, 2], in0=xa[:, 1:4, 0:NVAL], in1=wq[:, 2]
                )
                nc.vector.tensor_mul(
                    out=me[:, 3], in0=xb[:, 1:4, 0:NVAL], in1=wq[:, 3]
                )
                # hard products: mu_k at partition p-1 (x row 4p-1), mv_k at p
                mh = mpool.tile([128, 4, NVAL], F16, tag="mh")
                nc.vector.tensor_mul(
                    out=mh[0:127, 0], in0=xa[0:127, 3, 0:NVAL], in1=wu[:, 0]
                )
                nc.vector.tensor_mul(
                    out=mh[0:127, 1], in0=xb[0:127, 3, 0:NVAL], in1=wu[:, 1]
                )
                nc.vector.tensor_mul(
                    out=mh[:, 2], in0=xa[:, 0, 0:NVAL], in1=wv[:, 0]
                )
                nc.vector.tensor_mul(
                    out=mh[:, 3], in0=xb[:, 0, 0:NVAL], in1=wv[:, 1]
                )
                # pair-sums on DVE halve the PE matmul count
                se = mpool.tile([128, 2, 3, NVAL], F16, tag="se")
                nc.vector.tensor_add(out=se[:, 0], in0=me[:, 0], in1=me[:, 1])
                nc.vector.tensor_add(out=se[:, 1], in0=me[:, 2], in1=me[:, 3])
                sha = mpool.tile([128, 2, NVAL], F16, tag="sha")
                nc.vector.tensor_add(
                    out=sha[0:127, 0], in0=mh[0:127, 0], in1=mh[0:127, 1]
                )
                nc.vector.tensor_add(out=sha[:, 1], in0=mh[:, 2], in1=mh[:, 3])

                # PSUM accumulate: bank r holds out rows 4p+r, cols 1..510
                acc = psum_pool.tile([128, 4, 512], F32)
                a0 = acc[:, 0, 0:NVAL]
                nc.tensor.matmul(a0, sh[0:127, :], sha[0:127, 0], start=True, stop=False)
                nc.tensor.matmul(a0, idz, sha[:, 1], start=False, stop=False)
                nc.tensor.matmul(a0, idz, bh, start=False, stop=True)
                for r in range(1, 4):
                    ar = acc[:, r, 0:NVAL]
                    nc.tensor.matmul(ar, id16, se[:, 0, r - 1], start=True, stop=False)
                    nc.tensor.matmul(ar, id16, se[:, 1, r - 1], start=False, stop=False)
                    nc.tensor.matmul(ar, id16, be[:, r - 1], start=False, stop=True)

                # evacuate + zero col 0, ship
                ot = opool.tile([128, R, OH], F32, tag="ot")
                nc.vector.memset(ot[:, :, 0:1], 0.0)
                nc.scalar.copy(out=ot[:, :, 1:OH], in_=acc[:, :, 0:NVAL])
                nc.gpsimd.dma_start(
                    out=bass.AP(o_d, img * OW * OH, [[R * OH, 127], [1, R * OH]]),
                    in_=ot[0:127, :, :],
                )
                nc.gpsimd.dma_start(
                    out=bass.AP(o_d, img * OW * OH + 508 * OH, [[1, 3 * OH]]),
                    in_=ot[127:128, 0:3, :],
                )

    nc.finalize()
    return nc


_CACHE = {}


def _get_nc():
    if "nc" not in _CACHE:
        _CACHE["nc"] = _build()
    return _CACHE["nc"]


def kernel(x, weights, bias):
    assert x.shape == (B, C, W, H) and x.dtype == np.float32
    nc = _get_nc()

    in_maps = []
    for i in range(N_CORES):
        shard = np.ascontiguousarray(
            x[i * B_PER_CORE : (i + 1) * B_PER_CORE].reshape(IMGS, W, H)
        )
        in_maps.append({"x": shard, "weights": weights, "bias": bias})

    trace = os.environ.get("BASS_TRACE") == "1"
    res = run_bass_kernel_spmd(
        nc, in_maps, core_ids=list(range(N_CORES)), trace=trace
    )
    kernel.last_exec_time_ns = res.exec_time_ns
    kernel.last_results = res

    out = np.empty((B, C, OW, OH), dtype=np.float32)
    for i in range(N_CORES):
        out[i * B_PER_CORE : (i + 1) * B_PER_CORE] = res.results[i]["out"].reshape(
            B_PER_CORE, C, OW, OH
        )
    # Row 0 / col 0 are zero by definition; enforce host-side.
    out[:, :, 0, :] = 0.0
    out[:, :, :, 0] = 0.0
    return out
